# revision 1
# baseline (speedup 1.0000x reference)
"""Row-scale kernel: C = diag(A) @ B  (scale row i of B by A[i]).

Full shapes: A [16384] f32, B [16384, 4096] f32 -> C [16384, 4096] f32.
Sharding: pure data parallel over rows, 2048 rows per core on 8 cores.

Per-core layout: rows are interleaved over partitions, row r = p*T + t
(p = partition 0..127, t = row-tile 0..15).  The per-tile scale vector
a_sb[:, t] is then a plain column of an A tile loaded with ONE contiguous
8 KiB DMA, and each B tile is a clean 2D DMA (16 KiB contiguous per
partition, 256 KiB partition stride).

Raw Bass (no Tile framework), explicit software pipeline:
  SP sequencer  : B-tile loads  (HWDGE qSP ring)
  DVE           : per-partition scale multiply (in place, TensorScalarPtr)
  ACT sequencer : C-tile stores (HWDGE qAct ring)

Perf shaping — soft read/write phase alternation.  Measured on trn2:
pure reads sustain ~352 GB/s/core and pure writes ~380 GB/s, but a
50/50 mix only ~325 GB/s.  So tiles are processed in bursts of S=12
(24 MiB): stores of burst i are gated until the R-burst i is nearly done
(all but LEAD=2 loads), and loads of burst i+1 until the W-burst i is
nearly done (all but LEAD2=3 stores), so HBM sees mostly-pure read and
write phases with just enough overlap to bridge the transitions.
Measured steady state ~193 us/core vs ~207 us for free mixing
(64 MiB traffic -> ~347 GB/s/core; ideal alternation bound ~184 us).

Correctness structure:
  - per-slot semaphores with at most ONE outstanding DMA per semaphore,
    so cumulative wait thresholds are race-free (each DMA's 16 per-engine
    completions can interleave across concurrent DMAs otherwise);
  - every instruction carries at most one embedded wait (walrus rejects
    multi-wait TensorScalar), extra waits are standalone sequencer waits;
  - the store engine drains all store semaphores before the end-of-kernel
    barrier, else the NEFF can "complete" with C writes still in flight
    (observed as flaky wrong results under back-to-back execution).

reps>1 repeats the body back-to-back inside one NEFF (bench-only: lets a
long NEFF isolate steady-state per-rep time from launch overhead);
thresholds are cumulative over the global tile index so re-execution of
the NEFF itself is also safe (the Bass preamble re-zeros all kernel
semaphores at every execution start).
"""

import os

import numpy as np

import concourse.bass as bass
import concourse.mybir as mybir
from concourse.bass_utils import run_bass_kernel_spmd

N = 16384
M = 4096
N_CORES = 8
ROWS = N // N_CORES  # 2048 rows per core
P = 128              # SBUF partitions
T = ROWS // P        # 16 row-tiles of [128, 4096] per core

K = 12               # pipeline slots = burst size (12 x 16 KiB per partition)
LEAD = 2             # R->W transition overlap, in tiles
LEAD2 = 3            # W->R transition overlap, in tiles

_nc_cache = {}
last_exec_time_ns = None


def _build_nc(reps=1):
    nc = bass.Bass("TRN2", debug=False)
    A = nc.declare_dram_parameter("A", [ROWS], mybir.dt.float32, isOutput=False)
    B = nc.declare_dram_parameter("B", [ROWS, M], mybir.dt.float32, isOutput=False)
    C = nc.declare_dram_parameter("C", [ROWS, M], mybir.dt.float32, isOutput=True)

    # row r = p*T + t  (p outer, t inner) -> einops "(p t)"
    A2 = A.rearrange("(p t) -> p t", p=P)          # [128, 16]
    B3 = B.rearrange("(p t) m -> p t m", p=P)      # [128, 16, 4096]
    C3 = C.rearrange("(p t) m -> p t m", p=P)

    a_sb = nc.alloc_sbuf_tensor("a_sb", [P, T], mybir.dt.float32).ap()
    work = nc.alloc_sbuf_tensor("work", [P, K * M], mybir.dt.float32).ap()

    def slot(k):
        return work[:, k * M : (k + 1) * M]

    lda = nc.alloc_semaphore("lda")
    vs = nc.alloc_semaphore("vs")
    ld = [nc.alloc_semaphore(f"ld{k}") for k in range(K)]
    st = [nc.alloc_semaphore(f"st{k}") for k in range(K)]

    S = K            # burst size
    G = reps * T     # global tile count; data tile = g % T, slot = g % K

    with nc.Block() as block:

        @block.sync
        def _(sync: bass.BassEngine):
            sync.dma_start(out=a_sb, in_=A2).then_inc(lda, 16)
            for g in range(G):
                t, k = g % T, g % K
                if g >= S and g % S == 0:
                    # phase shaping: most of previous W-burst done
                    sync.wait_ge(st[S - 1 - LEAD2], 16 * (g // K))
                if g >= K:
                    # slot free once store g-K fully landed
                    sync.wait_ge(st[k], 16 * (g // K))
                sync.dma_start(out=slot(k), in_=B3[:, t, :]).then_inc(ld[k], 16)

        @block.vector
        def _(vector: bass.BassEngine):
            vector.wait_ge(lda, 16)
            for g in range(G):
                t, k = g % T, g % K
                vector.wait_ge(ld[k], 16 * (g // K + 1))
                vector.tensor_scalar_mul(slot(k), slot(k), a_sb[:, t : t + 1]).then_inc(
                    vs, 1
                )

        @block.scalar
        def _(scalar: bass.BassEngine):
            for g in range(G):
                t, k = g % T, g % K
                if g % S == 0:
                    # phase shaping: most of this R-burst done (threshold
                    # clamped for a partial final burst)
                    kk = S - 1 - LEAD
                    limit = min(g + S, G)
                    n_loads = ((limit - 1 - kk) // K + 1) if kk < limit else 0
                    if n_loads:
                        scalar.wait_ge(ld[kk], 16 * n_loads)
                scalar.wait_ge(vs, g + 1)
                scalar.dma_start(out=C3[:, t, :], in_=slot(k)).then_inc(st[k], 16)
            # drain: all C writes must land before the end-of-kernel barrier
            for k in range(K):
                scalar.wait_ge(st[k], 16 * ((G - 1 - k) // K + 1))

    return nc


def kernel(A, B):
    global last_exec_time_ns
    A = np.ascontiguousarray(np.asarray(A), dtype=np.float32)
    B = np.ascontiguousarray(np.asarray(B), dtype=np.float32)
    assert A.shape == (N,) and B.shape == (N, M)

    if "nc" not in _nc_cache:
        _nc_cache["nc"] = _build_nc()
    nc = _nc_cache["nc"]

    in_maps = [
        {"A": A[c * ROWS : (c + 1) * ROWS], "B": B[c * ROWS : (c + 1) * ROWS]}
        for c in range(N_CORES)
    ]
    trace = bool(os.environ.get("BASS_KERNEL_TRACE"))
    res = run_bass_kernel_spmd(nc, in_maps, list(range(N_CORES)), trace=trace)
    last_exec_time_ns = res.exec_time_ns
    return np.concatenate([res.results[c]["C"] for c in range(N_CORES)], axis=0)

